# revision 24
# baseline (speedup 1.0000x reference)
"""Mistral attention (B=2, S=2048, D=4096, H=32, KVH=8, HD=128) on 8 trn2 cores.

Sharding: core c -> (batch b = c//4, head-group g = c%4).
Each core computes q/k/v projections for its 8 Q heads + 2 KV heads of one
batch, RoPE, causal attention, and a row-parallel partial o_proj. The
partial o_proj output is produced TRANSPOSED ([D, S]); the host transposes
and sums the 4 partials per batch. No collectives.

All matmul operands are bf16 (PSUM accumulation stays fp32); rel err vs the
fp32 reference is ~5e-3. bf16 matters on trn2 because fp32r matmuls embed
their weight load serially in each instruction, while bf16 emits separate
LDWEIGHTS that the PE's reorder window hides, plus automatic fast-weight-
load.

Attention runs in transposed orientation: scoresT[keys, qtok] with keys on
partitions, so softmax uses an unstable exp (logits are O(10); fp32 exp-
safe), AV^T produces attn_out^T which feeds o_proj as the moving operand,
and the softmax denominator comes from elementwise bf16 accumulation of the
exp tiles (DVE/GpSimd) reduced by two deferred ones-matmuls per (head,
qblock). Scores for two adjacent key blocks share one 2-bank PSUM tile so
exp runs as one [128,1024] activation.

The attention phase is Act-engine-paced (exp ~2.3us per key-block pair vs
~1.9us of PE matmuls), and the PE executes in order, so the causal variant
SOFTWARE-PIPELINES the emission: the projection matmul groups of token
block t+1 (and the o_proj groups at the end) are emitted interleaved
between the attention groups of token block t. The PE then always has
dependency-ready work while exp drains, instead of stalling on PSUM-buffer
reuse.
"""

import os
import sys
from functools import partial

for _p in ("/opt/trn_rl_repo",):
    if _p not in sys.path:
        sys.path.insert(0, _p)

import numpy as np
from ml_dtypes import bfloat16

import concourse.bass as bass
import concourse.tile as tile
from concourse import bacc, mybir
from concourse.bass_utils import run_bass_kernel_spmd

F32 = mybir.dt.float32
BF16 = mybir.dt.bfloat16
EXP = mybir.ActivationFunctionType.Exp

B, S, D = 2, 2048, 4096
H, KVH, HD = 32, 8, 128
SCALE = HD ** -0.5
NCORES = 8

QH = H // 4              # 8 q heads per core
QCOLS = QH * HD          # 1024
KCOLS = (KVH // 4) * HD  # 256 (2 kv heads per core)
TOK = S
NCH = D // 128           # 32 contraction chunks

NEG = -1e9

_PROGRAMS = {}


def _build_program(variant: str):
    """variant: 'causal' | 'zero' | 'general'"""
    nc = bacc.Bacc("TRN2", target_bir_lowering=False, debug=False)

    hT = nc.dram_tensor("hT", [4, 2, 128, 16 * 512], BF16, kind="ExternalInput").ap()
    wq = nc.dram_tensor("wq", [8, 128, NCH * 128], BF16, kind="ExternalInput").ap()
    wk = nc.dram_tensor("wk", [128, 2 * NCH * 128], BF16, kind="ExternalInput").ap()
    wv = nc.dram_tensor("wv", [128, NCH * 256], BF16, kind="ExternalInput").ap()
    wo = nc.dram_tensor("wo", [32, 128, QH * 128], BF16, kind="ExternalInput").ap()
    cosT = nc.dram_tensor("cosT", [HD, TOK], BF16, kind="ExternalInput").ap()
    sinTr = nc.dram_tensor("sinTr", [HD, TOK], BF16, kind="ExternalInput").ap()
    ones = nc.dram_tensor("ones", [128, 1], BF16, kind="ExternalInput").ap()
    if variant == "causal":
        maskT = nc.dram_tensor("maskT", [128, 4 * 512], BF16, kind="ExternalInput").ap()
    elif variant == "general":
        maskT = nc.dram_tensor("maskT", [S, S], BF16, kind="ExternalInput").ap()
    else:
        maskT = None
    outT = nc.dram_tensor("outT", [D, TOK], BF16, kind="ExternalOutput").ap()

    if variant != "causal":
        qT_spill = nc.dram_tensor("qT_spill", [QCOLS, TOK], BF16).ap()

    NTH = 4
    THW = TOK // NTH         # 512

    with tile.TileContext(nc) as tc:
        with tc.tile_pool(name="per", bufs=1) as per, \
             tc.tile_pool(name="wrk", bufs=2) as wrk, \
             tc.tile_pool(name="one", bufs=1) as one, \
             tc.tile_pool(name="ps", bufs=2, space="PSUM") as psp:

            ones_sb = per.tile([128, 1], BF16, tag="ones")
            nc.sync.dma_start(ones_sb[:], ones[:])
            # resident K/V weights, chunked DMAs so first matmuls start early
            wk_sb = per.tile([128, 2 * NCH * 128], BF16, tag="wk")
            for j in range(4):
                nc.sync.dma_start(
                    wk_sb[:, j * 2048:(j + 1) * 2048], wk[:, j * 2048:(j + 1) * 2048])
            wv_sb = per.tile([128, NCH * 256], BF16, tag="wv")
            for j in range(4):
                nc.sync.dma_start(
                    wv_sb[:, j * 2048:(j + 1) * 2048], wv[:, j * 2048:(j + 1) * 2048])
            kT_sb = per.tile([HD, 2 * TOK], BF16, tag="kT")
            V_sb = per.tile([128, (TOK // 128) * KCOLS], BF16, tag="V")
            if variant == "causal":
                mask_sb = per.tile([128, 4 * 512], BF16, tag="mask")
                nc.sync.dma_start(mask_sb[:, :1024], maskT[:, :1024])
                nc.sync.dma_start(mask_sb[:, 1024:], maskT[:, 1024:])

            attn_sb = [one.tile([128, TOK], BF16, tag=f"at{h}", name=f"attn_{h}")
                       for h in range(QH)]

            # ------- projection work, split into interleavable pieces -------
            # kind "mm": pure PE matmuls / DMA issues, safe to emit between
            # attention key-block steps. kind "mx": touches DVE or Act
            # (rope, copies) -- emitted only at attention group boundaries so
            # those queues never delay the mask-add -> exp chain mid-group.
            def make_proj_pieces(th, st):
                ts = th * THW
                pieces = []

                def c_load():
                    hts = []
                    for j in range(8):
                        t = one.tile([128, 4 * THW], BF16, tag=f"hT{j}",
                                     name=f"hts_{th}_{j}")
                        half, jj = divmod(j, 4)
                        nc.sync.dma_start(
                            t[:, :1024],
                            hT[th, half, :, jj * 2048:jj * 2048 + 1024])
                        nc.sync.dma_start(
                            t[:, 1024:],
                            hT[th, half, :, jj * 2048 + 1024:(jj + 1) * 2048])
                        hts.append(t)
                    cos_t = wrk.tile([HD, THW], BF16, tag="cos", name=f"cos_{th}")
                    sin_t = wrk.tile([HD, THW], BF16, tag="sin", name=f"sin_{th}")
                    nc.sync.dma_start(cos_t[:], cosT[:, ts:ts + THW])
                    nc.sync.dma_start(sin_t[:], sinTr[:, ts:ts + THW])
                    st["hts"] = hts
                    st["cos"], st["sin"] = cos_t, sin_t
                    st["qT_lo"] = one.tile([128, 4 * 512], BF16, tag="qTbl",
                                           bufs=2, name=f"qlo_{th}")
                    st["qT_hi"] = one.tile([128, 4 * 512], BF16, tag="qTbh",
                                           bufs=2, name=f"qhi_{th}")
                pieces.append(("mm", c_load))

                def rope(ps, dst):
                    # out = x*cos + swap_halves(x)*sin_signed
                    m1 = wrk.tile([128, THW], F32, tag="m1")
                    nc.vector.tensor_mul(m1[:, :THW], ps[:], st["cos"][:])
                    m2 = wrk.tile([128, THW], F32, tag="m2")
                    nc.vector.tensor_mul(m2[0:64, :], ps[64:128, :],
                                         st["sin"][0:64, :])
                    nc.vector.tensor_mul(m2[64:128, :], ps[0:64, :],
                                         st["sin"][64:128, :])
                    nc.vector.tensor_add(dst, m1[:, :THW], m2[:])

                def add_gemm(name, stationary_of, moving_of, out_shape,
                             out_sl, epilogue):
                    cst = {}

                    def open_piece():
                        cst["ps"] = psp.tile(out_shape, F32, tag="pa",
                                             name=name)
                    for p0 in range(0, NCH, 8):
                        def mms(p0=p0, first=(p0 == 0)):
                            if first:
                                open_piece()
                            ps = cst["ps"]
                            for ic in range(p0, p0 + 8):
                                nc.tensor.matmul(
                                    ps[out_sl] if out_sl else ps[:],
                                    stationary_of(ic),
                                    moving_of(ic),
                                    start=(ic == 0), stop=(ic == NCH - 1))
                        pieces.append(("mm", mms))
                    pieces.append(("mx", lambda: epilogue(cst["ps"])))

                def c_k(cb):
                    add_gemm(
                        f"kp_{th}_{cb}",
                        lambda ic: wk_sb[:, (cb * NCH + ic) * 128:
                                         (cb * NCH + ic + 1) * 128],
                        lambda ic: st["hts"][ic // 4][:, (ic % 4) * THW:
                                                      (ic % 4 + 1) * THW],
                        [128, THW], None,
                        lambda ps: rope(
                            ps, kT_sb[:, cb * TOK + ts: cb * TOK + ts + THW]))

                def c_v(tc4):
                    # natural orientation: stationary = hidden chunks, moving
                    # = wv rows -> V with key tokens on partitions (no
                    # transpose needed for the AV matmul)
                    tb = th * 4 + tc4
                    add_gemm(
                        f"vp_{th}_{tc4}",
                        lambda ic: st["hts"][ic // 4][
                            :, (ic % 4) * THW + tc4 * 128:
                            (ic % 4) * THW + (tc4 + 1) * 128],
                        lambda ic: wv_sb[:, ic * 256:(ic + 1) * 256],
                        [128, 512], (slice(None), slice(0, 256)),
                        lambda ps: nc.scalar.copy(
                            V_sb[:, tb * KCOLS:(tb + 1) * KCOLS], ps[:, :256]))

                def c_q(cb):
                    wst = {}

                    def dma_w():
                        w_sb = wrk.tile([128, NCH * 128], BF16, tag="w",
                                        name=f"wq_{th}_{cb}")
                        nc.sync.dma_start(w_sb[:, :2048], wq[cb, :, :2048])
                        nc.sync.dma_start(w_sb[:, 2048:], wq[cb, :, 2048:])
                        wst["w"] = w_sb
                    pieces.append(("mm", dma_w))
                    qdst = "qT_lo" if cb < 4 else "qT_hi"
                    add_gemm(
                        f"qp_{th}_{cb}",
                        lambda ic: wst["w"][:, ic * 128:(ic + 1) * 128],
                        lambda ic: st["hts"][ic // 4][:, (ic % 4) * THW:
                                                      (ic % 4 + 1) * THW],
                        [128, THW], None,
                        lambda ps, q=qdst, cb=cb: rope(
                            ps,
                            st[q][:, (cb % 4) * 512:(cb % 4 + 1) * 512]))

                c_k(0)
                c_k(1)
                for i in range(4):
                    c_v(i)
                for i in range(QH):
                    c_q(i)
                return pieces

            # ---------------- o_proj pieces (one oc of one qcp) -------------
            # out^T[oc*128:+128, :] = sum_h wo[h, oc]^T @ attnT[h]
            def oproj_pieces(qcp, oc, out):
                ost = {}

                def dma_w():
                    wo_sb = wrk.tile([128, QH * 128], BF16, tag="wo",
                                     name=f"wo_{oc}_{qcp}")
                    nc.sync.dma_start(wo_sb[:], wo[oc])
                    ost["w"] = wo_sb
                out.append(("mm", dma_w))

                def mms(h0):
                    if h0 == 0:
                        ost["ps"] = psp.tile([128, 1024], F32, tag="pb",
                                             name=f"o_{oc}_{qcp}")
                    for hc in range(h0, h0 + 4):
                        for qh in range(2):
                            qc = qcp * 2 + qh
                            nc.tensor.matmul(
                                ost["ps"][:, qh * 512:(qh + 1) * 512],
                                ost["w"][:, hc * 128:(hc + 1) * 128],
                                attn_sb[hc][:, qc * 512:(qc + 1) * 512],
                                start=(hc == 0), stop=(hc == QH - 1))
                out.append(("mm", partial(mms, 0)))
                out.append(("mm", partial(mms, 4)))

                def fin():
                    ot = wrk.tile([128, 1024], BF16, tag="ot", bufs=3,
                                  name=f"ot_{oc}_{qcp}")
                    nc.scalar.copy(ot[:], ost["ps"][:])
                    nc.sync.dma_start(
                        outT[oc * 128:(oc + 1) * 128,
                             qcp * 1024:(qcp + 1) * 1024],
                        ot[:])
                out.append(("mx", fin))

            # Deferred softmax normalizations: each attention group's
            # denominator reduction + normalize is emitted at the end of the
            # NEXT group, so the PE never waits in-order on the DVE/GpSimd
            # exp-accumulation chain. Entries: (h, qb, attU_sb, acc_tile).
            norm_pending = []

            def _emit_norm(ent):
                h, qb, attU, acc = ent
                qs = qb * 512
                sum_ps = psp.tile([1, 512], F32, tag="pa", name=f"sum_{h}_{qb}")
                nc.tensor.matmul(sum_ps[:], ones_sb[:], acc[:, :512],
                                 start=True, stop=False)
                nc.tensor.matmul(sum_ps[:], ones_sb[:], acc[:, 512:],
                                 start=False, stop=True)
                recip = wrk.tile([1, 512], F32, tag="rcp", name=f"rcp_{h}_{qb}")
                nc.vector.reciprocal_approx_fast(recip[:], sum_ps[:])
                rb = wrk.tile([128, 512], F32, tag="m2", name=f"rb_{h}_{qb}")
                nc.gpsimd.partition_broadcast(rb[:], recip[:])
                nc.vector.tensor_mul(
                    attn_sb[h][:, qs:qs + 512], attU[:], rb[:])

            def flush_norm():
                for ent in norm_pending:
                    _emit_norm(ent)
                norm_pending.clear()

            def attention_group(hs, qb, qT_aps, filler=None):
                """Zipped scoresT/softmax/AV^T for q-head pair hs, query block
                qb. Key blocks are processed in pairs sharing one 2-bank PSUM
                tile so exp is a single [128,1024] activation. AV matmuls run
                one pair-step behind scores so the scalar engine's exp has a
                full pipeline step of lead time."""
                qs = qb * 512
                nkbp = 2 * qb + 2 if variant == "causal" else TOK // 256
                att_ps = [psp.tile([128, 512], F32, tag="att",
                                   name=f"att_{h}_{qb}") for h in hs]
                accs = [None, None]

                def co_of(kb):
                    if variant == "causal" and kb > 4 * qb:
                        return (kb - 4 * qb) * 128
                    return 0

                def emit_av(kbp, exps):
                    kb0, kb1 = 2 * kbp, 2 * kbp + 1
                    co0, co1 = co_of(kb0), co_of(kb1)
                    first = kbp == 0
                    last = kbp == nkbp - 1
                    for i, h in enumerate(hs):
                        kv = h // (QH // 2)
                        expT = exps[i]
                        nc.tensor.matmul(
                            att_ps[i][:, co0:],
                            V_sb[:, kb0 * KCOLS + kv * 128: kb0 * KCOLS + (kv + 1) * 128],
                            expT[:, co0:512],
                            start=first, stop=False)
                        nc.tensor.matmul(
                            att_ps[i][:, co1:],
                            V_sb[:, kb1 * KCOLS + kv * 128: kb1 * KCOLS + (kv + 1) * 128],
                            expT[:, 512 + co1:],
                            start=False, stop=last)

                pend = None
                for kbp in range(nkbp):
                    kb0, kb1 = 2 * kbp, 2 * kbp + 1
                    diag = variant == "causal" and kbp >= 2 * qb
                    if variant == "general":
                        mt = wrk.tile([128, 1024], BF16, tag="mt",
                                      name=f"mt_{qb}_{kbp}_{hs[0]}")
                        nc.sync.dma_start(
                            mt[:, :512], maskT[kb0 * 128:(kb0 + 1) * 128, qs:qs + 512])
                        nc.sync.dma_start(
                            mt[:, 512:], maskT[kb1 * 128:(kb1 + 1) * 128, qs:qs + 512])
                    exps = []
                    for i, h in enumerate(hs):
                        kv = h // (QH // 2)
                        eng = nc.gpsimd
                        s_w = psp.tile([128, 1024], F32, tag="pb",
                                       name=f"s_{h}_{qb}_{kbp}")
                        nc.tensor.matmul(
                            s_w[:, :512],
                            kT_sb[:, kv * TOK + kb0 * 128: kv * TOK + (kb0 + 1) * 128],
                            qT_aps[i][:],
                            start=True, stop=True)
                        nc.tensor.matmul(
                            s_w[:, 512:],
                            kT_sb[:, kv * TOK + kb1 * 128: kv * TOK + (kb1 + 1) * 128],
                            qT_aps[i][:],
                            start=True, stop=True)
                        if diag:
                            j = kbp - 2 * qb
                            msk = wrk.tile([128, 1024], F32, tag="msk",
                                           name=f"msk_{h}_{qb}_{kbp}")
                            nc.vector.tensor_add(
                                msk[:], s_w[:],
                                mask_sb[:, j * 1024:(j + 1) * 1024])
                            exp_in = msk
                        elif variant == "general":
                            msk = wrk.tile([128, 1024], F32, tag="msk",
                                           name=f"mskg_{h}_{qb}_{kbp}")
                            nc.vector.tensor_add(msk[:], s_w[:], mt[:])
                            exp_in = msk
                        else:
                            exp_in = s_w
                        expT = wrk.tile([128, 1024], BF16, tag="expT", bufs=4,
                                        name=f"exp_{h}_{qb}_{kbp}")
                        nc.scalar.activation(
                            expT[:], exp_in[:], EXP, scale=float(SCALE))
                        exps.append(expT)
                        if accs[i] is None:
                            accs[i] = expT
                        else:
                            if kbp == nkbp - 1:
                                tag = "accF"
                            else:
                                tag = "accA" if kbp % 2 else "accB"
                            nacc = wrk.tile([128, 1024], BF16, tag=tag, bufs=2,
                                            name=f"acc_{h}_{qb}_{kbp}")
                            eng.tensor_add(nacc[:], accs[i][:], expT[:])
                            accs[i] = nacc
                    if pend is not None:
                        emit_av(*pend)
                    pend = (kbp, exps)
                    if filler is not None:
                        filler()
                emit_av(*pend)
                prev = list(norm_pending)
                norm_pending.clear()
                for i, h in enumerate(hs):
                    attU = wrk.tile([128, 512], BF16, tag="attU", bufs=4,
                                    name=f"attU_{h}_{qb}")
                    nc.scalar.copy(attU[:], att_ps[i][:])
                    norm_pending.append((h, qb, attU, accs[i]))
                # flush the PREVIOUS group's normalization here: its exp
                # accumulation chain had this whole group's duration to
                # finish, so the ones-matmuls below never stall the PE
                for ent in prev:
                    _emit_norm(ent)

            # ======================= emission schedule ======================
            if variant == "causal":
                from collections import deque
                states = [dict() for _ in range(NTH)]
                for _, fn in make_proj_pieces(0, states[0]):
                    fn()
                for th in range(NTH):
                    st = states[th]
                    if th < NTH - 1:
                        nxt = deque(make_proj_pieces(th + 1, states[th + 1]))
                    else:
                        opp = []
                        for oc in range(32):
                            oproj_pieces(0, oc, opp)
                        nxt = deque(opp)
                    total = len(nxt)
                    popped = [0]

                    def pop_mm(n):
                        k = 0
                        while nxt and k < n and nxt[0][0] == "mm":
                            nxt.popleft()[1]()
                            k += 1
                        popped[0] += k

                    def pop_any(n):
                        k = 0
                        while nxt and k < n:
                            nxt.popleft()[1]()
                            k += 1
                        popped[0] += k

                    for gi, hp in enumerate(range(0, QH, 2)):
                        qsrc = st["qT_lo"] if hp < 4 else st["qT_hi"]
                        attention_group(
                            [hp, hp + 1], th,
                            [qsrc[:, (hp % 4) * 512:(hp % 4 + 1) * 512],
                             qsrc[:, (hp % 4 + 1) * 512:(hp % 4 + 2) * 512]],
                            filler=lambda: pop_mm(2))
                        pop_any(total * (gi + 1) // 4 - popped[0])
                    pop_any(len(nxt))
                flush_norm()
                tail = []
                for oc in range(32):
                    oproj_pieces(1, oc, tail)
                for _, fn in tail:
                    fn()
            else:
                for th in range(NTH):
                    st = {}
                    for _, fn in make_proj_pieces(th, st):
                        fn()
                    ts = th * THW
                    for qi, qt in ((0, st["qT_lo"]), (1, st["qT_hi"])):
                        nc.sync.dma_start(
                            qT_spill[qi * 512:(qi + 1) * 512, ts:ts + THW]
                            .rearrange("(i p) t -> p i t", p=128),
                            qt[:].rearrange("p (i t) -> p i t", i=4),
                        )
                for hp in range(0, QH, 2):
                    for qb in range(4):
                        qts = []
                        for h in (hp, hp + 1):
                            qT_t = wrk.tile([128, 512], BF16, tag="qTs",
                                            name=f"qt_{h}_{qb}")
                            nc.sync.dma_start(
                                qT_t[:],
                                qT_spill[h * 128:(h + 1) * 128,
                                         qb * 512:(qb + 1) * 512])
                            qts.append(qT_t)
                        attention_group([hp, hp + 1], qb, qts)
                flush_norm()
                for qcp in range(2):
                    chunk = []
                    for oc in range(32):
                        oproj_pieces(qcp, oc, chunk)
                    for _, fn in chunk:
                        fn()

    nc.compile()
    return nc


def _get_program(variant: str):
    if variant not in _PROGRAMS:
        _PROGRAMS[variant] = _build_program(variant)
    return _PROGRAMS[variant]


def _detect_variant(mask: np.ndarray) -> str:
    m = mask.reshape(mask.shape[-2], mask.shape[-1])
    if not m.any():
        return "zero"
    causal = np.where(
        np.tril(np.ones((S, S), dtype=bool)), np.float32(0.0), np.float32(NEG))
    if np.array_equal(m, causal):
        return "causal"
    return "general"


def kernel(hidden_states, cos, sin, attention_mask, Wq, Wk, Wv, Wo):
    hidden_states = np.asarray(hidden_states, dtype=np.float32)
    cos = np.asarray(cos, dtype=np.float32)
    sin = np.asarray(sin, dtype=np.float32)
    attention_mask = np.asarray(attention_mask, dtype=np.float32)
    Wq = np.asarray(Wq, dtype=np.float32)
    Wk = np.asarray(Wk, dtype=np.float32)
    Wv = np.asarray(Wv, dtype=np.float32)
    Wo = np.asarray(Wo, dtype=np.float32)

    variant = _detect_variant(attention_mask)
    nc = _get_program(variant)

    ones = np.ones((128, 1), dtype=bfloat16)

    if variant == "causal":
        i = np.arange(128)[:, None]
        j = np.arange(512)[None, :]
        strips = [
            np.where(i <= j - o * 128, np.float32(0.0), np.float32(NEG / SCALE))
            for o in range(4)
        ]
        maskT = np.concatenate(strips, axis=1).astype(bfloat16)
    elif variant == "general":
        m = attention_mask.reshape(S, S)
        maskT = np.ascontiguousarray(m.T / np.float32(SCALE)).astype(bfloat16)
    else:
        maskT = None

    per_batch = {}
    for b in range(B):
        sT = np.ascontiguousarray(sin[b].T)
        sinTr = np.concatenate([-sT[:64], sT[64:]], axis=0)
        hid = hidden_states[b]  # [2048, 4096]
        hT_t = np.ascontiguousarray(
            hid.reshape(4, 512, 2, 16, 128).transpose(0, 2, 4, 3, 1)
            .reshape(4, 2, 128, 16 * 512)).astype(bfloat16)
        per_batch[b] = (hT_t, np.ascontiguousarray(cos[b].T).astype(bfloat16),
                        np.ascontiguousarray(sinTr).astype(bfloat16))

    in_maps = []
    for c in range(NCORES):
        b, g = divmod(c, 4)
        hT_t, cosT_a, sinTr_a = per_batch[b]
        wq_c = Wq[:, g * QCOLS:(g + 1) * QCOLS]       # [4096, 1024]
        wq_t = np.ascontiguousarray(
            wq_c.reshape(NCH, 128, 8, 128).transpose(2, 1, 0, 3)
            .reshape(8, 128, NCH * 128)).astype(bfloat16)
        wk_c = Wk[:, g * KCOLS:(g + 1) * KCOLS]       # [4096, 256]
        wk_t = np.ascontiguousarray(
            wk_c.reshape(NCH, 128, 2, 128).transpose(1, 2, 0, 3)
            .reshape(128, 2 * NCH * 128)).astype(bfloat16)
        wv_c = Wv[:, g * KCOLS:(g + 1) * KCOLS]       # [4096, 256]
        wv_t = np.ascontiguousarray(
            wv_c.reshape(NCH, 128, 256).transpose(1, 0, 2)
            .reshape(128, NCH * 256)).astype(bfloat16)
        wo_c = Wo[g * QCOLS:(g + 1) * QCOLS, :]       # [1024, 4096]
        wo_t = np.ascontiguousarray(
            wo_c.reshape(8, 128, 32, 128).transpose(2, 1, 0, 3)
            .reshape(32, 128, 8 * 128)).astype(bfloat16)
        im = {
            "hT": hT_t,
            "wq": wq_t,
            "wk": wk_t,
            "wv": wv_t,
            "wo": wo_t,
            "cosT": cosT_a,
            "sinTr": sinTr_a,
            "ones": ones,
        }
        if maskT is not None:
            im["maskT"] = maskT
        in_maps.append(im)

    trace = bool(os.environ.get("KERNEL_TRACE"))
    res = run_bass_kernel_spmd(nc, in_maps, core_ids=list(range(NCORES)),
                               trace=trace)
    if trace:
        print(f"HW exec time: {res.exec_time_ns} ns")

    out = np.empty((B, S, D), dtype=np.float32)
    for b in range(B):
        acc = np.zeros((S, D), dtype=np.float64)
        for g in range(4):
            acc += res.results[4 * b + g]["outT"].astype(np.float32).T
        out[b] = acc.astype(np.float32)
    return out


# revision 26
# speedup vs baseline: 1.1077x; 1.1077x over previous
"""Mistral attention (B=2, S=2048, D=4096, H=32, KVH=8, HD=128) on 8 trn2 cores.

Sharding: core c -> (batch b = c//4, head-group g = c%4).
Each core computes q/k/v projections for its 8 Q heads + 2 KV heads of one
batch, RoPE, causal attention, and a row-parallel partial o_proj. The
partial o_proj output is produced TRANSPOSED ([D, S]); the host transposes
and sums the 4 partials per batch. No collectives.

All matmul operands are bf16 (PSUM accumulation stays fp32); rel err vs the
fp32 reference is ~5e-3. bf16 matters on trn2 because fp32r matmuls embed
their weight load serially in each instruction (~50-200ns/instr), while
bf16 emits separate LDWEIGHTS that the PE's 64-deep reorder window hides,
plus automatic fast-weight-load.

Attention runs in transposed orientation: scoresT[keys, qtok] with keys on
partitions, so softmax uses an unstable exp (logits are O(10) here; fp32
exp-safe), the key-sum is a ones-matmul, and AV^T produces attn_out^T which
feeds o_proj directly as the moving operand. Causal attention for query
block t is fused right after the projections of token block t. Scores for
two adjacent key blocks share one 2-bank PSUM tile so exp runs as one
[128,1024] activation (halves the ~293ns Act init cost per instruction).
The per-block ones-matmul key-sums double as PE-side filler that paces the
tensor engine to the scalar engine's exp throughput.
"""

import os
import sys

for _p in ("/opt/trn_rl_repo",):
    if _p not in sys.path:
        sys.path.insert(0, _p)

import numpy as np
from ml_dtypes import bfloat16

import concourse.bass as bass
import concourse.tile as tile
from concourse import bacc, mybir
from concourse.bass_utils import run_bass_kernel_spmd

F32 = mybir.dt.float32
BF16 = mybir.dt.bfloat16
EXP = mybir.ActivationFunctionType.Exp

B, S, D = 2, 2048, 4096
H, KVH, HD = 32, 8, 128
SCALE = HD ** -0.5
NCORES = 8

QH = H // 4              # 8 q heads per core
QCOLS = QH * HD          # 1024
KCOLS = (KVH // 4) * HD  # 256 (2 kv heads per core)
TOK = S
NCH = D // 128           # 32 contraction chunks

NEG = -1e9

_PROGRAMS = {}


def _build_program(variant: str):
    """variant: 'causal' | 'zero' | 'general'"""
    nc = bacc.Bacc("TRN2", target_bir_lowering=False, debug=False)

    hT = nc.dram_tensor("hT", [4, 2, 128, 16 * 512], BF16, kind="ExternalInput").ap()
    wq = nc.dram_tensor("wq", [8, 128, NCH * 128], BF16, kind="ExternalInput").ap()
    wk = nc.dram_tensor("wk", [128, 2 * NCH * 128], BF16, kind="ExternalInput").ap()
    wv = nc.dram_tensor("wv", [128, NCH * 256], BF16, kind="ExternalInput").ap()
    wo = nc.dram_tensor("wo", [32, 128, QH * 128], BF16, kind="ExternalInput").ap()
    cosT = nc.dram_tensor("cosT", [HD, TOK], F32, kind="ExternalInput").ap()
    sinTr = nc.dram_tensor("sinTr", [HD, TOK], F32, kind="ExternalInput").ap()
    ones = nc.dram_tensor("ones", [128, 1], BF16, kind="ExternalInput").ap()
    if variant == "causal":
        maskT = nc.dram_tensor("maskT", [128, 4 * 512], F32, kind="ExternalInput").ap()
    elif variant == "general":
        maskT = nc.dram_tensor("maskT", [S, S], F32, kind="ExternalInput").ap()
    else:
        maskT = None
    outT = nc.dram_tensor("outT", [D, TOK], BF16, kind="ExternalOutput").ap()

    if variant != "causal":
        qT_spill = nc.dram_tensor("qT_spill", [QCOLS, TOK], BF16).ap()

    NTH = 4
    THW = TOK // NTH         # 512

    with tile.TileContext(nc) as tc:
        with tc.tile_pool(name="per", bufs=1) as per, \
             tc.tile_pool(name="wrk", bufs=2) as wrk, \
             tc.tile_pool(name="one", bufs=1) as one, \
             tc.tile_pool(name="ps", bufs=2, space="PSUM") as psp:

            ones_sb = per.tile([128, 1], BF16, tag="ones")
            nc.sync.dma_start(ones_sb[:], ones[:])
            # resident K/V weights, chunked DMAs so first matmuls start early
            wk_sb = per.tile([128, 2 * NCH * 128], BF16, tag="wk")
            for j in range(4):
                nc.sync.dma_start(
                    wk_sb[:, j * 2048:(j + 1) * 2048], wk[:, j * 2048:(j + 1) * 2048])
            wv_sb = per.tile([128, NCH * 256], BF16, tag="wv")
            for j in range(4):
                nc.sync.dma_start(
                    wv_sb[:, j * 2048:(j + 1) * 2048], wv[:, j * 2048:(j + 1) * 2048])
            kT_sb = per.tile([HD, 2 * TOK], BF16, tag="kT")
            V_sb = per.tile([128, (TOK // 128) * KCOLS], BF16, tag="V")
            if variant == "causal":
                mask_sb = per.tile([128, 4 * 512], F32, tag="mask")
                nc.sync.dma_start(mask_sb[:, :1024], maskT[:, :1024])
                nc.sync.dma_start(mask_sb[:, 1024:], maskT[:, 1024:])

            attn_sb = [one.tile([128, TOK], BF16, tag=f"at{h}", name=f"attn_{h}")
                       for h in range(QH)]

            def attention_group(hs, qb, qT_aps):
                """Zipped scoresT/softmax/AV^T for q-head pair hs, query block
                qb. Key blocks are processed in pairs sharing one 2-bank PSUM
                tile so exp is a single [128,1024] activation. AV/sum matmuls
                run one pair-step behind scores so the scalar engine's exp has
                a full pipeline step of lead time."""
                qs = qb * 512
                nkbp = 2 * qb + 2 if variant == "causal" else TOK // 256
                att_ps = [psp.tile([128, 512], F32, tag="pa", name=f"att_{h}_{qb}")
                          for h in hs]
                sum_ps = [psp.tile([1, 512], F32, tag="sum", name=f"sum_{h}_{qb}")
                          for h in hs]

                def co_of(kb):
                    if variant == "causal" and kb > 4 * qb:
                        return (kb - 4 * qb) * 128
                    return 0

                def emit_av(kbp, exps):
                    kb0, kb1 = 2 * kbp, 2 * kbp + 1
                    co0, co1 = co_of(kb0), co_of(kb1)
                    first = kbp == 0
                    last = kbp == nkbp - 1
                    for i, h in enumerate(hs):
                        kv = h // (QH // 2)
                        expT = exps[i]
                        nc.tensor.matmul(
                            att_ps[i][:, co0:],
                            V_sb[:, kb0 * KCOLS + kv * 128: kb0 * KCOLS + (kv + 1) * 128],
                            expT[:, co0:512],
                            start=first, stop=False)
                        nc.tensor.matmul(
                            att_ps[i][:, co1:],
                            V_sb[:, kb1 * KCOLS + kv * 128: kb1 * KCOLS + (kv + 1) * 128],
                            expT[:, 512 + co1:],
                            start=False, stop=last)
                    for i, h in enumerate(hs):
                        expT = exps[i]
                        nc.tensor.matmul(
                            sum_ps[i][:, co0:], ones_sb[:], expT[:, co0:512],
                            start=first, stop=False)
                        nc.tensor.matmul(
                            sum_ps[i][:, co1:], ones_sb[:], expT[:, 512 + co1:],
                            start=False, stop=last)

                pend = None
                for kbp in range(nkbp):
                    kb0, kb1 = 2 * kbp, 2 * kbp + 1
                    co0, co1 = co_of(kb0), co_of(kb1)
                    diag = variant == "causal" and kbp >= 2 * qb
                    if variant == "general":
                        mt = wrk.tile([128, 1024], F32, tag="mt",
                                      name=f"mt_{qb}_{kbp}_{hs[0]}")
                        nc.sync.dma_start(
                            mt[:, :512], maskT[kb0 * 128:(kb0 + 1) * 128, qs:qs + 512])
                        nc.sync.dma_start(
                            mt[:, 512:], maskT[kb1 * 128:(kb1 + 1) * 128, qs:qs + 512])
                    exps = []
                    for i, h in enumerate(hs):
                        kv = h // (QH // 2)
                        s_w = psp.tile([128, 1024], F32, tag="pb",
                                       name=f"s_{h}_{qb}_{kbp}")
                        nc.tensor.matmul(
                            s_w[:, co0:512],
                            kT_sb[:, kv * TOK + kb0 * 128: kv * TOK + (kb0 + 1) * 128],
                            qT_aps[i][:, co0:],
                            start=True, stop=True)
                        nc.tensor.matmul(
                            s_w[:, 512 + co1:],
                            kT_sb[:, kv * TOK + kb1 * 128: kv * TOK + (kb1 + 1) * 128],
                            qT_aps[i][:, co1:],
                            start=True, stop=True)
                        if diag:
                            j = kbp - 2 * qb
                            msk = wrk.tile([128, 1024], F32, tag="msk",
                                           name=f"msk_{h}_{qb}_{kbp}")
                            nc.vector.tensor_add(
                                msk[:, co0:], s_w[:, co0:],
                                mask_sb[:, j * 1024 + co0:(j + 1) * 1024])
                            exp_in, ci = msk, co0
                        elif variant == "general":
                            msk = wrk.tile([128, 1024], F32, tag="msk",
                                           name=f"mskg_{h}_{qb}_{kbp}")
                            nc.vector.tensor_add(msk[:], s_w[:], mt[:])
                            exp_in, ci = msk, 0
                        else:
                            exp_in, ci = s_w, 0
                        expT = wrk.tile([128, 1024], BF16, tag="expT", bufs=4,
                                        name=f"exp_{h}_{qb}_{kbp}")
                        nc.scalar.activation(
                            expT[:, ci:], exp_in[:, ci:], EXP, scale=float(SCALE))
                        exps.append(expT)
                    if pend is not None:
                        emit_av(*pend)
                    pend = (kbp, exps)
                emit_av(*pend)
                for i, h in enumerate(hs):
                    recip = wrk.tile([1, 512], F32, tag="rcp", name=f"rcp_{h}_{qb}")
                    nc.vector.reciprocal_approx_fast(recip[:], sum_ps[i][:])
                    rb = wrk.tile([128, 512], F32, tag="m2", name=f"rb_{h}_{qb}")
                    nc.gpsimd.partition_broadcast(rb[:], recip[:])
                    nc.vector.tensor_mul(
                        attn_sb[h][:, qs:qs + 512], att_ps[i][:], rb[:])

            # ============ Phase A (+fused attention for causal) ============
            for th in range(NTH):
                ts = th * THW
                # hidden^T block [D, 512] as 8 sub-tiles of 4 D-chunks
                hts = []
                for j in range(8):
                    t = one.tile([128, 4 * THW], BF16, tag=f"hT{j}")
                    half, jj = divmod(j, 4)
                    nc.sync.dma_start(
                        t[:, :1024], hT[th, half, :, jj * 2048:jj * 2048 + 1024])
                    nc.sync.dma_start(
                        t[:, 1024:], hT[th, half, :, jj * 2048 + 1024:(jj + 1) * 2048])
                    hts.append(t)
                cos_t = wrk.tile([HD, THW], F32, tag="cos")
                sin_t = wrk.tile([HD, THW], F32, tag="sin")
                nc.sync.dma_start(cos_t[:], cosT[:, ts:ts + THW])
                nc.sync.dma_start(sin_t[:], sinTr[:, ts:ts + THW])

                qT_lo = one.tile([128, 4 * 512], BF16, tag="qTbl")
                qT_hi = one.tile([128, 4 * 512], BF16, tag="qTbh")

                def rope(ps, dst):
                    # out = x*cos + swap_halves(x)*sin_signed
                    m1 = wrk.tile([128, THW], F32, tag="m1")
                    nc.vector.tensor_mul(m1[:, :THW], ps[:], cos_t[:])
                    m2 = wrk.tile([128, THW], F32, tag="m2")
                    nc.vector.tensor_mul(m2[0:64, :], ps[64:128, :], sin_t[0:64, :])
                    nc.vector.tensor_mul(m2[64:128, :], ps[0:64, :], sin_t[64:128, :])
                    nc.vector.tensor_add(dst, m1[:, :THW], m2[:])

                # K projection (stationary = resident wk chunks)
                for cb in range(2):
                    ps = psp.tile([128, THW], F32, tag="pa", name=f"kp_{th}_{cb}")
                    for ic in range(NCH):
                        nc.tensor.matmul(
                            ps[:],
                            wk_sb[:, (cb * NCH + ic) * 128:(cb * NCH + ic + 1) * 128],
                            hts[ic // 4][:, (ic % 4) * THW:(ic % 4 + 1) * THW],
                            start=(ic == 0), stop=(ic == NCH - 1))
                    rope(ps, kT_sb[:, cb * TOK + ts: cb * TOK + ts + THW])

                # V projection, natural orientation (stationary = hidden chunks,
                # moving = wv rows) -> V with key tokens on partitions, no
                # transpose needed for the AV matmul.
                for tc4 in range(4):
                    v_ps = psp.tile([128, 512], F32, tag="pa", name=f"vp_{th}_{tc4}")
                    for ic in range(NCH):
                        nc.tensor.matmul(
                            v_ps[:, :256],
                            hts[ic // 4][:, (ic % 4) * THW + tc4 * 128:
                                         (ic % 4) * THW + (tc4 + 1) * 128],
                            wv_sb[:, ic * 256:(ic + 1) * 256],
                            start=(ic == 0), stop=(ic == NCH - 1))
                    tb = th * 4 + tc4
                    nc.scalar.copy(
                        V_sb[:, tb * KCOLS:(tb + 1) * KCOLS], v_ps[:, :256])

                # Q projection (stationary = streamed wq chunks)
                for cb in range(QH):
                    w_sb = wrk.tile([128, NCH * 128], BF16, tag="w",
                                    name=f"wq_{th}_{cb}")
                    nc.sync.dma_start(w_sb[:, :2048], wq[cb, :, :2048])
                    nc.sync.dma_start(w_sb[:, 2048:], wq[cb, :, 2048:])
                    ps = psp.tile([128, THW], F32, tag="pa", name=f"qp_{th}_{cb}")
                    for ic in range(NCH):
                        nc.tensor.matmul(
                            ps[:],
                            w_sb[:, ic * 128:(ic + 1) * 128],
                            hts[ic // 4][:, (ic % 4) * THW:(ic % 4 + 1) * THW],
                            start=(ic == 0), stop=(ic == NCH - 1))
                    qdst = qT_lo if cb < 4 else qT_hi
                    rope(ps, qdst[:, (cb % 4) * 512:(cb % 4 + 1) * 512])

                if variant == "causal":
                    for hp in range(0, QH, 2):
                        qsrc = qT_lo if hp < 4 else qT_hi
                        attention_group(
                            [hp, hp + 1], th,
                            [qsrc[:, (hp % 4) * 512:(hp % 4 + 1) * 512],
                             qsrc[:, (hp % 4 + 1) * 512:(hp % 4 + 2) * 512]])
                else:
                    for qi, qt in ((0, qT_lo), (1, qT_hi)):
                        nc.sync.dma_start(
                            qT_spill[qi * 512:(qi + 1) * 512, ts:ts + THW]
                            .rearrange("(i p) t -> p i t", p=128),
                            qt[:].rearrange("p (i t) -> p i t", i=4),
                        )

            if variant != "causal":
                for hp in range(0, QH, 2):
                    for qb in range(4):
                        qts = []
                        for h in (hp, hp + 1):
                            qT_t = wrk.tile([128, 512], BF16, tag="qTs",
                                            name=f"qt_{h}_{qb}")
                            nc.sync.dma_start(
                                qT_t[:],
                                qT_spill[h * 128:(h + 1) * 128,
                                         qb * 512:(qb + 1) * 512])
                            qts.append(qT_t)
                        attention_group([hp, hp + 1], qb, qts)

            # ============ Phase C: o_proj partial, transposed out ============
            # out^T[oc*128:+128, :] = sum_h wo[h, oc]^T @ attnT[h]
            for oc in range(32):
                wo_sb = wrk.tile([128, QH * 128], BF16, tag="wo", name=f"wo_{oc}")
                nc.sync.dma_start(wo_sb[:], wo[oc])
                for qcp in range(2):
                    o_ps = psp.tile([128, 1024], F32, tag="pb",
                                    name=f"o_{oc}_{qcp}")
                    for hc in range(QH):
                        for qh in range(2):
                            qc = qcp * 2 + qh
                            nc.tensor.matmul(
                                o_ps[:, qh * 512:(qh + 1) * 512],
                                wo_sb[:, hc * 128:(hc + 1) * 128],
                                attn_sb[hc][:, qc * 512:(qc + 1) * 512],
                                start=(hc == 0), stop=(hc == QH - 1))
                    ot = wrk.tile([128, 1024], BF16, tag="ot", bufs=3,
                                  name=f"ot_{oc}_{qcp}")
                    nc.scalar.copy(ot[:], o_ps[:])
                    nc.sync.dma_start(
                        outT[oc * 128:(oc + 1) * 128,
                             qcp * 1024:(qcp + 1) * 1024],
                        ot[:])

    nc.compile()
    return nc


def _get_program(variant: str):
    if variant not in _PROGRAMS:
        _PROGRAMS[variant] = _build_program(variant)
    return _PROGRAMS[variant]


def _detect_variant(mask: np.ndarray) -> str:
    m = mask.reshape(mask.shape[-2], mask.shape[-1])
    if not m.any():
        return "zero"
    causal = np.where(
        np.tril(np.ones((S, S), dtype=bool)), np.float32(0.0), np.float32(NEG))
    if np.array_equal(m, causal):
        return "causal"
    return "general"


def kernel(hidden_states, cos, sin, attention_mask, Wq, Wk, Wv, Wo):
    hidden_states = np.asarray(hidden_states, dtype=np.float32)
    cos = np.asarray(cos, dtype=np.float32)
    sin = np.asarray(sin, dtype=np.float32)
    attention_mask = np.asarray(attention_mask, dtype=np.float32)
    Wq = np.asarray(Wq, dtype=np.float32)
    Wk = np.asarray(Wk, dtype=np.float32)
    Wv = np.asarray(Wv, dtype=np.float32)
    Wo = np.asarray(Wo, dtype=np.float32)

    variant = _detect_variant(attention_mask)
    nc = _get_program(variant)

    ones = np.ones((128, 1), dtype=bfloat16)

    if variant == "causal":
        i = np.arange(128)[:, None]
        j = np.arange(512)[None, :]
        strips = [
            np.where(i <= j - o * 128, np.float32(0.0), np.float32(NEG / SCALE))
            for o in range(4)
        ]
        maskT = np.concatenate(strips, axis=1).astype(np.float32)
    elif variant == "general":
        m = attention_mask.reshape(S, S)
        maskT = np.ascontiguousarray(m.T / np.float32(SCALE))
    else:
        maskT = None

    per_batch = {}
    for b in range(B):
        sT = np.ascontiguousarray(sin[b].T)
        sinTr = np.concatenate([-sT[:64], sT[64:]], axis=0)
        hid = hidden_states[b]  # [2048, 4096]
        hT_t = np.ascontiguousarray(
            hid.reshape(4, 512, 2, 16, 128).transpose(0, 2, 4, 3, 1)
            .reshape(4, 2, 128, 16 * 512)).astype(bfloat16)
        per_batch[b] = (hT_t, np.ascontiguousarray(cos[b].T),
                        np.ascontiguousarray(sinTr))

    in_maps = []
    for c in range(NCORES):
        b, g = divmod(c, 4)
        hT_t, cosT_a, sinTr_a = per_batch[b]
        wq_c = Wq[:, g * QCOLS:(g + 1) * QCOLS]       # [4096, 1024]
        wq_t = np.ascontiguousarray(
            wq_c.reshape(NCH, 128, 8, 128).transpose(2, 1, 0, 3)
            .reshape(8, 128, NCH * 128)).astype(bfloat16)
        wk_c = Wk[:, g * KCOLS:(g + 1) * KCOLS]       # [4096, 256]
        wk_t = np.ascontiguousarray(
            wk_c.reshape(NCH, 128, 2, 128).transpose(1, 2, 0, 3)
            .reshape(128, 2 * NCH * 128)).astype(bfloat16)
        wv_c = Wv[:, g * KCOLS:(g + 1) * KCOLS]       # [4096, 256]
        wv_t = np.ascontiguousarray(
            wv_c.reshape(NCH, 128, 256).transpose(1, 0, 2)
            .reshape(128, NCH * 256)).astype(bfloat16)
        wo_c = Wo[g * QCOLS:(g + 1) * QCOLS, :]       # [1024, 4096]
        wo_t = np.ascontiguousarray(
            wo_c.reshape(8, 128, 32, 128).transpose(2, 1, 0, 3)
            .reshape(32, 128, 8 * 128)).astype(bfloat16)
        im = {
            "hT": hT_t,
            "wq": wq_t,
            "wk": wk_t,
            "wv": wv_t,
            "wo": wo_t,
            "cosT": cosT_a,
            "sinTr": sinTr_a,
            "ones": ones,
        }
        if maskT is not None:
            im["maskT"] = maskT
        in_maps.append(im)

    trace = bool(os.environ.get("KERNEL_TRACE"))
    res = run_bass_kernel_spmd(nc, in_maps, core_ids=list(range(NCORES)),
                               trace=trace)
    if trace:
        print(f"HW exec time: {res.exec_time_ns} ns")

    out = np.empty((B, S, D), dtype=np.float32)
    for b in range(B):
        acc = np.zeros((S, D), dtype=np.float64)
        for g in range(4):
            acc += res.results[4 * b + g]["outT"].astype(np.float32).T
        out[b] = acc.astype(np.float32)
    return out
